# revision 28
# baseline (speedup 1.0000x reference)
"""Encoder layer (pre-norm attention + MLP) on 8 Trainium2 cores.

Sharding: core = (batch b in 0..3, half hf in 0..1). Each core receives the
full 2048-token sequence of batch b, transposed to [E, S] and rolled so the
core's own 1024 tokens are columns 0:1024 (attention and LN are invariant to
key order). The core computes K/V over the full sequence and everything else
only for its own tokens. No collectives; the host reassembles the 8 shards.

Design notes (vs the DRAM-roundtrip v1):
- K/Q/V, fc1 activations and all intermediates stay in SBUF; the only DRAM
  traffic is x (once), weights (fp16), and the final fused output.
- All 16-bit tensors are fp16 (not bf16): same PE rate and size, 4 more
  mantissa bits, so the accuracy stays near the f32 baseline.
- QKV projection runs per head-pair, pipelined with attention for the
  previous pair, so dense projection matmuls fill the exp-bound gaps of
  attention and keep the PE HAM clock un-throttled at 2.4 GHz.
- The softmax 1/denominator broadcast runs on the idle GpSimd engine
  (a PE ones-matmul here stalls the tensor engine on the DVE reciprocal).
- Scalar engine only runs exp during the attention region; Sqrt (LN stats)
  and Gelu (applied straight from PSUM) happen outside it, so activation
  tables load ~3 times total instead of thrashing.
- fc2 output, residual and bias are fused on-chip: one output tensor.
"""

import numpy as np
import ml_dtypes
from contextlib import ExitStack

import concourse.bacc as bacc
import concourse.mybir as mybir
import concourse.tile as tile
from concourse.bass_utils import run_bass_kernel_spmd

F32 = mybir.dt.float32
F32R = mybir.dt.float32r
F16 = mybir.dt.float16
AF = mybir.ActivationFunctionType
OP = mybir.AluOpType

B, S, E, H, D, FF = 4, 2048, 1024, 16, 64, 4096
TOWN = 1024  # tokens owned per core
ET = E // 128  # 8
FT = FF // 128  # 32
NT = S // 128  # 16 token tiles (full seq)
HP = H // 2  # 8 head pairs
NCORES = 8
EPS = 1e-6


def _build():
    nc = bacc.Bacc()

    x_t = nc.dram_tensor("x_t", [E, S], F32R, kind="ExternalInput")
    # weights pre-tiled on host: [out_tile, 128(part=e%128), e_tile, out_in_tile]
    wq_t = nc.dram_tensor("wq_t", [ET, 128, ET, 128], F16, kind="ExternalInput")
    wk_t = nc.dram_tensor("wk_t", [ET, 128, ET, 128], F16, kind="ExternalInput")
    wv_t = nc.dram_tensor("wv_t", [2, 128, ET, 512], F16, kind="ExternalInput")
    qb = nc.dram_tensor("qb", [128, ET], F32, kind="ExternalInput")
    kb = nc.dram_tensor("kb", [128, ET], F32, kind="ExternalInput")
    vb = nc.dram_tensor("vb", [E], F32R, kind="ExternalInput")
    wout_t = nc.dram_tensor("wout_t", [ET, 128, ET, 128], F16,
                            kind="ExternalInput")
    ob = nc.dram_tensor("ob", [128, ET], F32, kind="ExternalInput")
    wfc1_t = nc.dram_tensor("wfc1_t", [FT, 128, ET, 128], F16,
                            kind="ExternalInput")
    f1b = nc.dram_tensor("f1b", [128, FT], F32, kind="ExternalInput")
    wfc2_t = nc.dram_tensor("wfc2_t", [ET, 128, FT, 128], F16,
                            kind="ExternalInput")
    f2b = nc.dram_tensor("f2b", [128, ET], F32, kind="ExternalInput")

    out_d = nc.dram_tensor("out_d", [E, TOWN], F32, kind="ExternalOutput")

    inv_e = 1.0 / E
    unb = float(E) / (E - 1.0)  # E/(E-1) for unbiased variance

    with tile.TileContext(nc) as tc, ExitStack() as ctx:
        consts = ctx.enter_context(tc.tile_pool(name="consts", bufs=1))
        ones_f32 = consts.tile([128, 256], F32)
        nc.vector.memset(ones_f32, 1.0)
        ones128 = consts.tile([128, 128], F32R)
        nc.vector.tensor_copy(ones128, ones_f32[:, 0:128])
        qb_sb = consts.tile([128, ET], F32)
        kb_sb = consts.tile([128, ET], F32)
        ob_sb = consts.tile([128, ET], F32)
        f1b_sb = consts.tile([128, FT], F32)
        f2b_sb = consts.tile([128, ET], F32)
        nc.sync.dma_start(out=qb_sb, in_=qb[:, :])
        nc.sync.dma_start(out=kb_sb, in_=kb[:, :])
        nc.sync.dma_start(out=ob_sb, in_=ob[:, :])
        nc.sync.dma_start(out=f1b_sb, in_=f1b[:, :])
        nc.sync.dma_start(out=f2b_sb, in_=f2b[:, :])
        # v bias broadcast across all partitions (v is token-major)
        vb_row = consts.tile([1, E], F32R)
        nc.sync.dma_start(out=vb_row, in_=vb[None, :])
        vb_bc = consts.tile([128, E], F32)
        with tc.tile_pool(name="vbb_p", bufs=2, space="PSUM") as vbb_p:
            for c in range(2):
                ps = vbb_p.tile([128, 512], F32, tag="vbbc")
                nc.tensor.matmul(ps, ones128[0:1, :],
                                 vb_row[:, c * 512:(c + 1) * 512],
                                 start=True, stop=True)
                nc.scalar.activation(vb_bc[:, c * 512:(c + 1) * 512], ps,
                                     AF.Copy)

        def ln_stats(src_of, n_chunks, csz, mean_bc, rstd_bc, xsq_pool,
                     misc_pool, ps_pool, tagp):
            # src_of(c) is [128, ET, csz] f32r; stats over the feature dim via
            # ones-matmul (result broadcast to all 128 partitions for free).
            for c in range(n_chunks):
                sl = slice(c * csz, (c + 1) * csz)
                src_c = src_of(c)
                ps_sum = ps_pool.tile([128, csz], F32, tag=tagp + "a")
                ps_ssq = ps_pool.tile([128, csz], F32, tag=tagp + "b")
                for a in range(ET):
                    xa = src_c[:, a, :]
                    xsq = xsq_pool.tile([128, csz], F32R, tag="xsq")
                    # squares on ACT (idle during LN passes; Square shares
                    # the Sqrt table set) — the DVE is the critical path.
                    nc.scalar.activation(xsq, xa, AF.Square)
                    nc.tensor.matmul(ps_sum, ones128, xa,
                                     start=(a == 0), stop=(a == ET - 1))
                    nc.tensor.matmul(ps_ssq, ones128, xsq,
                                     start=(a == 0), stop=(a == ET - 1))
                m = misc_pool.tile([128, csz], F32, tag="m")
                nc.vector.tensor_scalar_mul(m, ps_sum, inv_e)
                nc.vector.tensor_copy(mean_bc[:, sl], m)
                msq = misc_pool.tile([128, csz], F32, tag="msq")
                nc.vector.tensor_tensor(msq, m, m, OP.mult)
                nc.vector.tensor_scalar_mul(msq, msq, unb)
                var = misc_pool.tile([128, csz], F32, tag="var")
                nc.vector.tensor_scalar(var, ps_ssq, 1.0 / (E - 1.0), None,
                                        OP.mult)
                nc.vector.tensor_tensor(var, var, msq, OP.subtract)
                std = misc_pool.tile([128, csz], F32, tag="std")
                nc.scalar.activation(std, var, AF.Sqrt)
                nc.vector.tensor_scalar_add(std, std, EPS)
                nc.vector.reciprocal(rstd_bc[:, sl], std)

        # Long-lived cross-stage tensors. Pools must close in LIFO order, so
        # open the longest-lived first: x2 (D..G), x_own (..D), ctxn (C..D),
        # z1 (A..C), v_sb (Bv..C).
        s_z1 = ExitStack()
        s_kv = ExitStack()
        s_ctxn = ExitStack()
        s_xo = ExitStack()
        s_x2 = ExitStack()
        s_z2 = ExitStack()
        s_h = ExitStack()

        px2 = s_x2.enter_context(tc.tile_pool(name="px2", bufs=1))
        x2 = px2.tile([128, ET, TOWN], F32R)
        # the core's own 1024 tokens of x = stage A's chunks 0 and 1; keep
        # those two chunk tiles alive until stage D's residual add instead
        # of re-loading x_own from DRAM.
        pxo = s_xo.enter_context(tc.tile_pool(name="pxo", bufs=1))
        xc01 = [pxo.tile([128, ET, 512], F32R, name=f"xc01_{c}")
                for c in range(2)]
        pctxn = s_ctxn.enter_context(tc.tile_pool(name="pctxn", bufs=1))
        ctxn = pctxn.tile([128, ET, TOWN], F16)
        # z1 lives as one tile per 512-token chunk so stages Bv/B_kq can
        # start on chunk 0 while stage A still normalizes later chunks
        # (Tile tracks dependencies per tile).
        CA = 512
        NCH = S // CA
        pz1 = s_z1.enter_context(tc.tile_pool(name="pz1", bufs=1))
        z1c = [pz1.tile([128, ET, CA], F16, name=f"z1_{c}")
               for c in range(NCH)]

        def z1m(a, csl):  # moving-operand slice of z1 for tokens csl
            c0, c1 = csl.start // CA, (csl.stop - 1) // CA
            assert c0 == c1
            return z1c[c0][:, a, slice(csl.start - c0 * CA,
                                       csl.stop - c0 * CA)]

        # ------------- Stage A: LN1 stats + z1 over full sequence ----------
        with tc.tile_pool(name="pa", bufs=2) as pa, \
             tc.tile_pool(name="pa_st", bufs=1) as pa_st, \
             tc.tile_pool(name="pa_xsq", bufs=3) as pa_xsq, \
             tc.tile_pool(name="pa_misc", bufs=2) as pa_misc, \
             tc.tile_pool(name="psA", bufs=2, space="PSUM") as psA:
            mean1 = pa_st.tile([128, S], F32)
            rstd1 = pa_st.tile([128, S], F32)
            xre = x_t.rearrange("(a p) s -> p a s", p=128)
            for c in range(NCH):
                sl = slice(c * CA, (c + 1) * CA)
                if c < 2:
                    xc = xc01[c]
                else:
                    xc = pa.tile([128, ET, CA], F32R, tag="xc", name=f"xc{c}")
                nc.sync.dma_start(out=xc, in_=xre[:, :, sl])
                ln_stats(lambda _: xc, 1, CA, mean1[:, sl], rstd1[:, sl],
                         pa_xsq, pa_misc, psA, "st")
                mb = mean1[:, None, sl].broadcast_to([128, ET, CA])
                rs = rstd1[:, None, sl].broadcast_to([128, ET, CA])
                nc.vector.tensor_tensor(z1c[c], xc, mb, OP.subtract)
                nc.vector.tensor_tensor(z1c[c], z1c[c], rs, OP.mult)

        pkv = s_kv.enter_context(tc.tile_pool(name="pkv", bufs=1))
        # [part = t%128, t_tile, head, 64 v dims + 1 ones col]
        v_sb = pkv.tile([128, NT, H, 65], F16)
        nc.vector.tensor_copy(
            v_sb[:, :, :, 64],
            ones_f32[:, 0:NT * H].rearrange("p (a b) -> p a b", a=NT))

        # ----------- Stage Bv: v projection (full seq, token-major) --------
        with tc.tile_pool(name="pbv_w", bufs=2) as pbw, \
             tc.tile_pool(name="psBv", bufs=2, space="PSUM") as psBv:
            for c in range(2):
                wv_c = pbw.tile([128, ET, 512], F16, tag="w")
                nc.sync.dma_start(out=wv_c, in_=wv_t[c])
                for tt in range(NT):
                    tsl = slice((tt % 4) * 128, (tt % 4 + 1) * 128)
                    ps = psBv.tile([128, 512], F32, tag="v",
                                   name=f"psv{tt}_{c}")
                    for a in range(ET):
                        nc.tensor.matmul(ps, z1c[tt // 4][:, a, tsl],
                                         wv_c[:, a, :],
                                         start=(a == 0), stop=(a == ET - 1))
                    nc.vector.tensor_tensor(
                        v_sb[:, tt, c * 8:(c + 1) * 8, 0:64],
                        ps.rearrange("p (h w) -> p h w", w=64),
                        vb_bc[:, c * 512:(c + 1) * 512].rearrange(
                            "p (h w) -> p h w", w=64),
                        OP.add)

        # ---- Stage B_kq + C: per head-pair QKV projection then attention --
        with tc.tile_pool(name="pc_w", bufs=3) as pcw, \
             tc.tile_pool(name="pc_kq", bufs=1) as pckq, \
             tc.tile_pool(name="pc_pr", bufs=3) as pcpr, \
             tc.tile_pool(name="pc_misc", bufs=2) as pcm, \
             tc.tile_pool(name="psC", bufs=1, space="PSUM") as psC:
            for hp in range(HP):
                wk_hp = pcw.tile([128, ET, 128], F16, tag="w",
                                 name=f"wk{hp}")
                nc.sync.dma_start(out=wk_hp, in_=wk_t[hp])
                wq_hp = pcw.tile([128, ET, 128], F16, tag="w",
                                 name=f"wq{hp}")
                nc.sync.dma_start(out=wq_hp, in_=wq_t[hp])
                k_pair = pckq.tile([128, S], F16, tag="k", bufs=2,
                                   name=f"kp{hp}")
                q_pair = pckq.tile([128, TOWN], F16, tag="q", bufs=2,
                                   name=f"qp{hp}")
                for c in range(4):
                    csl = slice(c * 512, (c + 1) * 512)
                    psk = psC.tile([128, 512], F32, tag="kq", bufs=2,
                                   name=f"psk{hp}_{c}")
                    for a in range(ET):
                        nc.tensor.matmul(psk, wk_hp[:, a, :], z1m(a, csl),
                                         start=(a == 0), stop=(a == ET - 1))
                    nc.vector.tensor_scalar(k_pair[:, csl], psk,
                                            kb_sb[:, hp:hp + 1], None, OP.add)
                for c in range(2):
                    csl = slice(c * 512, (c + 1) * 512)
                    psq = psC.tile([128, 512], F32, tag="kq", bufs=2,
                                   name=f"psq{hp}_{c}")
                    for a in range(ET):
                        nc.tensor.matmul(psq, wq_hp[:, a, :], z1m(a, csl),
                                         start=(a == 0), stop=(a == ET - 1))
                    nc.vector.tensor_scalar(q_pair[:, csl], psq,
                                            qb_sb[:, hp:hp + 1], None, OP.add)
                # attention per head; matmul operands address partitions
                # 64:128 directly via tile_position for the odd head.
                for qc in range(2):
                    qsl = slice(qc * 512, (qc + 1) * 512)
                    for hh in range(2):
                        h = 2 * hp + hh
                        lo = 64 * hh
                        kh = k_pair[lo:lo + 64, :]
                        qh = q_pair[lo:lo + 64, qsl]
                        ctx_ps = psC.tile([65, 512], F32, tag="ctx", bufs=2,
                                          name=f"ctx{h}_{qc}")
                        for k2 in range(NT // 2):
                            s_ps = psC.tile([128, 2, 512], F32, tag="s",
                                            bufs=2, name=f"s{h}_{qc}_{k2}")
                            for j in range(2):
                                kt = 2 * k2 + j
                                nc.tensor.matmul(
                                    s_ps[:, j, :],
                                    kh[:, kt * 128:(kt + 1) * 128],
                                    qh, start=True, stop=True)
                            pr = pcpr.tile([128, 2, 512], F16, tag="pr")
                            nc.scalar.activation(pr, s_ps, AF.Exp, scale=0.125)
                            for j in range(2):
                                kt = 2 * k2 + j
                                nc.tensor.matmul(
                                    ctx_ps, v_sb[:, kt, h, :], pr[:, j, :],
                                    start=(kt == 0), stop=(kt == NT - 1))
                        rec = pcm.tile([1, 512], F32R, tag="rec")
                        with nc.allow_low_precision(
                                reason="f32r rounding of softmax denom"):
                            nc.vector.reciprocal(rec, ctx_ps[64:65, :])
                        # broadcast the per-token 1/denom to 64 partitions on
                        # the otherwise-idle GpSimd engine (a PE matmul here
                        # stalls the tensor engine on the recip result).
                        rb = pcm.tile([64, 512], F32R, tag="rb")
                        nc.gpsimd.partition_broadcast(rb, rec)
                        nc.vector.tensor_tensor(
                            ctxn[lo:lo + 64, hp, qsl],
                            ctx_ps[0:64, :], rb, OP.mult)
        s_kv.close()   # v_sb dead after attention
        s_z1.close()   # z1 dead after last head-pair projection

        # ------------- Stage D: out-proj + residual ------------------------
        with tc.tile_pool(name="pd_w", bufs=2) as pdw, \
             tc.tile_pool(name="pd_ev", bufs=4) as pde, \
             tc.tile_pool(name="psD", bufs=1, space="PSUM") as psD:
            for ot in range(ET):
                w_ot = pdw.tile([128, ET, 128], F16, tag="w")
                nc.sync.dma_start(out=w_ot, in_=wout_t[ot])
                pss = [psD.tile([128, 512], F32, tag="p", bufs=4,
                                name=f"psd{ot}_{c}") for c in range(2)]
                for a in range(ET):
                    for c in range(2):
                        nc.tensor.matmul(
                            pss[c], w_ot[:, a, :],
                            ctxn[:, a, c * 512:(c + 1) * 512],
                            start=(a == 0), stop=(a == ET - 1))
                for c in range(2):
                    csl = slice(c * 512, (c + 1) * 512)
                    ev = pde.tile([128, 512], F32, tag="ev")
                    nc.scalar.activation(ev, pss[c], AF.Identity,
                                         bias=ob_sb[:, ot:ot + 1])
                    nc.vector.tensor_tensor(
                        x2[:, ot, csl], ev, xc01[c][:, ot, :], OP.add)
        s_ctxn.close()
        s_xo.close()

        ph = s_h.enter_context(tc.tile_pool(name="ph", bufs=1))
        h_sb = ph.tile([128, FT, TOWN], F16)
        pz2 = s_z2.enter_context(tc.tile_pool(name="pz2", bufs=1))
        z2 = pz2.tile([128, ET, TOWN], F16)

        # ------------- Stage E: LN2 stats + z2 -----------------------------
        with tc.tile_pool(name="pe_st", bufs=1) as pe_st, \
             tc.tile_pool(name="pe_tmp", bufs=3) as pe_tmp, \
             tc.tile_pool(name="psE", bufs=1, space="PSUM") as psE:
            mean2 = pe_st.tile([128, TOWN], F32)
            rstd2 = pe_st.tile([128, TOWN], F32)
            ln_stats(lambda c: x2[:, :, c * 512:(c + 1) * 512],
                     2, 512, mean2, rstd2, pe_tmp, pe_tmp, psE, "s2")
            for c in range(2):
                sl = slice(c * 512, (c + 1) * 512)
                mb = mean2[:, None, sl].broadcast_to([128, ET, 512])
                rs = rstd2[:, None, sl].broadcast_to([128, ET, 512])
                nc.vector.tensor_tensor(z2[:, :, sl], x2[:, :, sl], mb,
                                        OP.subtract)
                nc.vector.tensor_tensor(z2[:, :, sl], z2[:, :, sl], rs,
                                        OP.mult)

            # --------- Stage F: fc1 + gelu (direct from PSUM) --------------
            with tc.tile_pool(name="pf_w", bufs=3) as pfw:
                for ft in range(FT):
                    w_ft = pfw.tile([128, ET, 128], F16, tag="w")
                    nc.sync.dma_start(out=w_ft, in_=wfc1_t[ft])
                    pss = [psE.tile([128, 512], F32, tag="p", bufs=4,
                                    name=f"psf{ft}_{c}") for c in range(2)]
                    for a in range(ET):
                        for c in range(2):
                            nc.tensor.matmul(
                                pss[c], w_ft[:, a, :],
                                z2[:, a, c * 512:(c + 1) * 512],
                                start=(a == 0), stop=(a == ET - 1))
                    for c in range(2):
                        csl = slice(c * 512, (c + 1) * 512)
                        nc.scalar.activation(h_sb[:, ft, csl], pss[c],
                                             AF.Gelu,
                                             bias=f1b_sb[:, ft:ft + 1])
        s_z2.close()

        # ------------- Stage G: fc2 + residual + bias -> out ---------------
        with tc.tile_pool(name="pg_w", bufs=2) as pgw, \
             tc.tile_pool(name="pg_ev", bufs=4) as pge, \
             tc.tile_pool(name="psG", bufs=1, space="PSUM") as psG:
            for ot in range(ET):
                w_ot = pgw.tile([128, FT, 128], F16, tag="w")
                nc.sync.dma_start(out=w_ot, in_=wfc2_t[ot])
                pss = [psG.tile([128, 512], F32, tag="p", bufs=4,
                                name=f"psg{ot}_{c}") for c in range(2)]
                for f in range(FT):
                    for c in range(2):
                        nc.tensor.matmul(
                            pss[c], w_ot[:, f, :],
                            h_sb[:, f, c * 512:(c + 1) * 512],
                            start=(f == 0), stop=(f == FT - 1))
                for c in range(2):
                    csl = slice(c * 512, (c + 1) * 512)
                    ev = pge.tile([128, 512], F32, tag="ev")
                    nc.scalar.activation(ev, pss[c], AF.Identity,
                                         bias=f2b_sb[:, ot:ot + 1])
                    osb = pge.tile([128, 512], F32, tag="osb")
                    nc.vector.tensor_tensor(osb, ev, x2[:, ot, csl], OP.add)
                    nc.sync.dma_start(
                        out=out_d[ot * 128:(ot + 1) * 128, csl], in_=osb)
        s_h.close()
        s_x2.close()

    nc.finalize()
    return nc


_NC_CACHE = {}


def _get_nc():
    if "v2" not in _NC_CACHE:
        _NC_CACHE["v2"] = _build()
    return _NC_CACHE["v2"]


def _tile_w(w_t, n_out_tiles, dtype=np.float16):
    # [E_in, O] (in-feature rows) -> [O//128, 128, E_in//128, 128] so each
    # output-tile's weight block is contiguous (multi-KB runs per partition).
    e_in, o = w_t.shape
    arr = w_t.reshape(e_in // 128, 128, n_out_tiles, o // n_out_tiles)
    return np.ascontiguousarray(arr.transpose(2, 1, 0, 3).astype(dtype))


def _prepare_in_maps(inputs):
    f = np.float32
    x = np.asarray(inputs["x"], f)
    w_qkv = np.asarray(inputs["w_qkv"], np.float64)
    ln1_w = np.asarray(inputs["ln1_w"], np.float64)
    ln1_b = np.asarray(inputs["ln1_b"], np.float64)
    ln2_w = np.asarray(inputs["ln2_w"], np.float64)
    ln2_b = np.asarray(inputs["ln2_b"], np.float64)
    w_fc1 = np.asarray(inputs["w_fc1"], np.float64)

    wqkv_s = (w_qkv * ln1_w[None, :])  # fold LN1 gamma
    qkv_bias = ln1_b @ np.asarray(inputs["w_qkv"], np.float64).T  # [3E]
    wqkv_t = np.ascontiguousarray(wqkv_s.T, f)  # [E, 3E]
    wq_t = _tile_w(wqkv_t[:, 0:E], ET)
    wk_t = _tile_w(wqkv_t[:, E:2 * E], ET)
    wv_t = _tile_w(wqkv_t[:, 2 * E:3 * E], 2)  # [2,128,ET,512] rhs chunks
    col = lambda v: np.ascontiguousarray(
        np.asarray(v, f).reshape(-1, 128).T)  # [o] -> [128, o//128]
    qb = col(qkv_bias[0:E])
    kb = col(qkv_bias[E:2 * E])
    vb = np.ascontiguousarray(qkv_bias[2 * E:3 * E], f)

    wout_t = _tile_w(np.ascontiguousarray(np.asarray(inputs["w_out"], f).T),
                     ET)
    ob = col(inputs["b_out"])

    wfc1_s = (w_fc1 * ln2_w[None, :])
    f1b_flat = np.asarray(inputs["b_fc1"], np.float64) + ln2_b @ w_fc1.T
    f1b = col(f1b_flat)
    wfc1_t = _tile_w(np.ascontiguousarray(wfc1_s.T, f), FT)
    wfc2_t = _tile_w(np.ascontiguousarray(np.asarray(inputs["w_fc2"], f).T),
                     ET)
    f2b = col(inputs["b_fc2"])

    shared = dict(wq_t=wq_t, wk_t=wk_t, wv_t=wv_t, qb=qb, kb=kb, vb=vb,
                  wout_t=wout_t, ob=ob, wfc1_t=wfc1_t, f1b=f1b,
                  wfc2_t=wfc2_t, f2b=f2b)
    in_maps = []
    for core in range(NCORES):
        b, hf = divmod(core, 2)
        xs = np.roll(x[b], -hf * TOWN, axis=0)  # own tokens first
        x_tc = np.ascontiguousarray(xs.T)  # [E, S]
        in_maps.append(dict(x_t=x_tc, **shared))
    return in_maps


def _assemble(inputs, results):
    f = np.float32
    out = np.empty((B, S, E), f)
    for core in range(NCORES):
        b, hf = divmod(core, 2)
        out[b, hf * TOWN:(hf + 1) * TOWN, :] = results[core]["out_d"].T
    return out


def run(inputs, **spmd_kwargs):
    nc = _get_nc()
    in_maps = _prepare_in_maps(inputs)
    res = run_bass_kernel_spmd(nc, in_maps, core_ids=list(range(NCORES)),
                               **spmd_kwargs)
    return _assemble(inputs, res.results), res


def kernel(**inputs):
    out, _ = run(inputs)
    return out
